# revision 7
# baseline (speedup 1.0000x reference)
"""Distributed Trainium2 attention kernel (8 NeuronCores).

Sharding: 4-way data parallel over batch x 2-way tensor parallel over heads.
Core c handles batch c//2 and head-group c%2 (8 of 16 heads). Host sums the
two row-parallel out-proj partials per batch.

Key perf structure (vs the earlier baseline):
- Single activation table (natural_log_exp): rms-norm uses exp(-0.5*ln(ms))
  instead of Sqrt, so the softmax Exp never triggers a mid-kernel
  ACT_TABLE_LOAD. Any >=2us PE stall risks a 655us HAM 4/8 down-clock window.
- Softmax exp split: columns [0:EXPA) on the Act engine (true Exp),
  [EXPA:1024) on the otherwise-idle DVE via a one-op Schraudolph fast-exp
  (fused y=A*x+B with int16 convert, bitcast to bf16; the bf16 staircase
  matches the Act path's bf16 output). Global scale bias cancels in softmax.
- Phase order: k/v projections for all 32 t-tiles, then q tiles 0-7, then
  attention chunks; q tiles 8-31 are processed inside the chunk loop (DMA,
  projection, rotary, transpose pipelined across head iterations) so their
  DVE work hides under attention matmuls.
- Softmax denominators ride as psum row 64 (ones column in vaug); normalize
  uses reciprocal_approx_fast (1 DVE op) on the gpsimd-broadcast row.
- PE warm-up bursts bridge the two spots where the PE would otherwise idle
  >1us (phase-2 entry DVE lag, attention->out-proj drain): idle windows
  trigger the HAM clock gate (observed 655.36us at half PE throughput).
"""
import sys
import os
from contextlib import ExitStack

if '/opt/trn_rl_repo' not in sys.path:
    sys.path.insert(0, '/opt/trn_rl_repo')

import numpy as np
import ml_dtypes

bf16 = ml_dtypes.bfloat16

T = 4096
D = 1024
HL = 8          # local heads per core
HD = 64
NT = T // 128   # 32 t-tiles
KT = D // 128   # 8 contraction tiles for projections
NCH = 4         # chunks of 1024 along t for attention
CW = 1024       # chunk width
PAIRS = 4       # head pairs per core
EPS = 1.1920928955078125e-07

EXPA = 768      # softmax columns per 1024 computed on Act engine (rest: DVE)
LN2 = 0.6931471805599453
# z = bitcast_bf16(int16(psc * SCHRA + SCHRB)) ~= exp(0.125 * psc) * const
SCHRA = 0.125 * (2.0 ** 23 / LN2) / 65536.0
SCHRB = (127.0 * 2.0 ** 23 - 485000.0) / 65536.0

# interleaved q-tile pipeline schedule (relative tile 0..7 within next chunk)
DMA_SCHED = {0: [0, 1, 2], 1: [3], 2: [4], 3: [5], 4: [6], 5: [7]}
PROJ_SCHED = {1: [0, 1], 2: [2], 3: [3], 4: [4], 5: [5], 6: [6, 7]}


def build():
    from concourse import bacc, tile, mybir

    BF16 = mybir.dt.bfloat16
    F32 = mybir.dt.float32
    I16 = mybir.dt.int16
    AF = mybir.ActivationFunctionType
    ALU = mybir.AluOpType
    AX = mybir.AxisListType

    nc = bacc.Bacc()
    xT = nc.declare_dram_parameter("xT", [D, T], BF16, isOutput=False)
    wqT = nc.declare_dram_parameter("wqT", [D, 512], BF16, isOutput=False)
    wkT = nc.declare_dram_parameter("wkT", [D, 512], BF16, isOutput=False)
    wvT = nc.declare_dram_parameter("wvT", [D, 512], BF16, isOutput=False)
    woT = nc.declare_dram_parameter("woT", [512, D], BF16, isOutput=False)
    cos2 = nc.declare_dram_parameter("cos2", [T, 64], BF16, isOutput=False)
    ss = nc.declare_dram_parameter("ss", [T, 64], BF16, isOutput=False)
    ident = nc.declare_dram_parameter("ident", [128, 128], BF16, isOutput=False)
    out = nc.declare_dram_parameter("out", [T, D], F32, isOutput=True)

    with tile.TileContext(nc) as tc:
        with tc.tile_pool(name="persist", bufs=1) as persist:
            qTc = [persist.tile([128, PAIRS, CW], BF16, tag=f"qT{c}",
                                name=f"qT{c}") for c in range(NCH)]
            kT = persist.tile([128, PAIRS, T], BF16, tag="kT")
            vaug = persist.tile([128, NT, HL, 65], BF16, tag="vaug")
            wo_sb = persist.tile([128, 4, D], BF16, tag="wo_sb")
            id_sb = persist.tile([128, 128], BF16, tag="id_sb")
            eps_t = persist.tile([128, 1], F32, tag="eps_t")
            yTn = persist.tile([128, PAIRS, T], BF16, tag="yTn")

            nc.vector.memset(vaug[:, :, :, 64:65], 1.0)
            nc.vector.memset(eps_t[:], EPS)

            # pools alive through phases A..C
            span = ExitStack()
            with span:
                wqp = span.enter_context(tc.tile_pool(name="wqp", bufs=1))
                xcolp = span.enter_context(tc.tile_pool(name="xcolp", bufs=3))
                cscr = span.enter_context(tc.tile_pool(name="cscr", bufs=2))
                csmall = span.enter_context(tc.tile_pool(name="csmall", bufs=2))
                ps_tr = span.enter_context(
                    tc.tile_pool(name="ps_tr", bufs=1, space="PSUM"))

                w_sb = {}
                w_sb["q"] = wqp.tile([128, KT, 512], BF16, tag="wq",
                                     name="w_q_sb")
                for ki in range(KT):
                    nc.sync.dma_start(
                        w_sb["q"][:, ki, :], wqT[ki * 128:(ki + 1) * 128, :])
                cos_sb = wqp.tile([128, NT, 64], BF16, tag="cos_sb")
                ss_sb = wqp.tile([128, NT, 64], BF16, tag="ss_sb")
                nc.sync.dma_start(
                    cos_sb[:], cos2[:].rearrange("(t p) d -> p t d", p=128))
                nc.sync.dma_start(
                    ss_sb[:], ss[:].rearrange("(t p) d -> p t d", p=128))
                nc.sync.dma_start(id_sb[:], ident[:])
                nc.sync.dma_start(
                    wo_sb[:], woT[:].rearrange("(k p) n -> p k n", p=128))

                # preload the natural_log_exp activation table before any
                # matmul work (Ln + Exp live in one table; no later ATL)
                dummy = csmall.tile([128, 1], F32, tag="dummy")
                nc.scalar.copy(dummy[:], eps_t[:])
                nc.scalar.activation(dummy[:], dummy[:], AF.Exp)
                nc.scalar.activation(dummy[:], dummy[:], AF.Ln, bias=eps_t[:])

                def rotary_rms(t, ps_q, store, tcol, pool_s, pool_m, on_act):
                    """rotary + rms-normalize one projected [128,512] tile;
                    returns the qn tile to transpose later.
                    on_act: route copies via Act (phase A/B) or DVE (phase C).
                    """
                    cp = nc.scalar.copy if on_act else nc.vector.tensor_copy
                    ctb = cos_sb[:, t, :].unsqueeze(1).broadcast_to(
                        [128, HL, 64])
                    stb = ss_sb[:, t, :].unsqueeze(1).broadcast_to(
                        [128, HL, 64])
                    qb = pool_s.tile([128, 512], BF16, tag="qb")
                    cp(qb[:], ps_q[:])
                    b3 = qb[:].rearrange("p (h u d) -> p h u d", h=HL, u=2)
                    qs = pool_s.tile([128, 512], BF16, tag="qs")
                    qs3 = qs[:].rearrange("p (h u d) -> p h u d", h=HL, u=2)
                    cp(qs3[:, :, 0, :], b3[:, :, 1, :])
                    cp(qs3[:, :, 1, :], b3[:, :, 0, :])
                    t1 = pool_s.tile([128, 512], BF16, tag="t1")
                    nc.vector.tensor_tensor(
                        t1[:].rearrange("p (h d) -> p h d", h=HL),
                        qb[:].rearrange("p (h d) -> p h d", h=HL),
                        ctb, op=ALU.mult)
                    r = pool_s.tile([128, 512], BF16, tag="r")
                    nc.vector.tensor_tensor(
                        r[:].rearrange("p (h d) -> p h d", h=HL),
                        qs[:].rearrange("p (h d) -> p h d", h=HL),
                        stb, op=ALU.mult)
                    nc.vector.tensor_tensor(r[:], t1[:], r[:], op=ALU.add)
                    sq = pool_s.tile([128, 512], BF16, tag="sq")
                    nc.gpsimd.tensor_tensor(sq[:], r[:], r[:], op=ALU.mult)
                    ms8 = pool_m.tile([128, HL], F32, tag="ms8")
                    nc.vector.tensor_reduce(
                        ms8[:], sq[:].rearrange("p (h d) -> p h d", h=HL),
                        axis=AX.X, op=ALU.add)
                    # 1/sqrt(ms+eps) = exp(-0.5*ln(ms/64 + eps)) -- stays in
                    # the same act table as the softmax Exp
                    lnv = pool_m.tile([128, HL], F32, tag="lnv")
                    nc.scalar.activation(
                        lnv[:], ms8[:], AF.Ln, scale=1.0 / HD, bias=eps_t[:])
                    rinv = pool_m.tile([128, HL], F32, tag="rinv")
                    nc.scalar.activation(rinv[:], lnv[:], AF.Exp, scale=-0.5)
                    qn = pool_s.tile([128, 512], BF16, tag="qn")
                    nc.vector.tensor_tensor(
                        qn[:].rearrange("p (h d) -> p h d", h=HL),
                        r[:].rearrange("p (h d) -> p h d", h=HL),
                        rinv[:].unsqueeze(2).broadcast_to([128, HL, 64]),
                        op=ALU.mult)
                    return (qn, store, tcol)

                def emit_tr(pend):
                    qn, store, tcol = pend
                    tp = ps_tr.tile([128, 4, 128], BF16, tag="tp")
                    for cb in range(4):
                        nc.tensor.transpose(
                            tp[:, cb, :], qn[:, cb * 128:(cb + 1) * 128],
                            id_sb[:])
                    nc.vector.tensor_copy(
                        store[:, :, tcol:tcol + 128], tp[:])

                def dma_xcol(t):
                    xcol = xcolp.tile([128, KT, 128], BF16, tag="xcol")
                    nc.sync.dma_start(
                        xcol[:],
                        xT[:, t * 128:(t + 1) * 128].rearrange(
                            "(k p) t -> p k t", p=128))
                    return xcol

                def proj(xcol, name, ps_pool):
                    ps = ps_pool.tile([128, 512], F32, tag="pqkv",
                                      name=f"ps_{name}")
                    for ki in range(KT):
                        nc.tensor.matmul(
                            ps[:], xcol[:, ki, :], w_sb[name][:, ki, :],
                            start=(ki == 0), stop=(ki == KT - 1))
                    return ps

                # ================= Phase A: k/v for all t =================
                phaseA = ExitStack()
                with phaseA:
                    wkv = phaseA.enter_context(
                        tc.tile_pool(name="wkv", bufs=1))
                    ascr = phaseA.enter_context(
                        tc.tile_pool(name="ascr", bufs=2))
                    asmall = phaseA.enter_context(
                        tc.tile_pool(name="asmall", bufs=2))
                    ps_qkv = phaseA.enter_context(
                        tc.tile_pool(name="ps_qkv", bufs=4, space="PSUM"))

                    for name, param in (("k", wkT), ("v", wvT)):
                        w_sb[name] = wkv.tile([128, KT, 512], BF16,
                                              tag=f"w{name}",
                                              name=f"w_{name}_sb")
                        for ki in range(KT):
                            nc.sync.dma_start(
                                w_sb[name][:, ki, :],
                                param[ki * 128:(ki + 1) * 128, :])

                    pend = None
                    for t in range(NT):
                        xcol = dma_xcol(t)
                        ps_k = proj(xcol, "k", ps_qkv)
                        ps_v = proj(xcol, "v", ps_qkv)
                        nc.scalar.copy(
                            vaug[:, t, :, 0:64],
                            ps_v[:].rearrange("p (h d) -> p h d", h=HL))
                        if pend is not None:
                            emit_tr(pend)
                        pend = rotary_rms(t, ps_k, kT, t * 128,
                                          ascr, asmall, on_act=True)
                    emit_tr(pend)

                    # ============= Phase B: q tiles 0..7 =============
                    pend = None
                    for t in range(8):
                        xcol = dma_xcol(t)
                        ps_q = proj(xcol, "q", ps_qkv)
                        if pend is not None:
                            emit_tr(pend)
                        pend = rotary_rms(t, ps_q, qTc[0], (t % 8) * 128,
                                          ascr, asmall, on_act=True)
                    # PE warm-up burst bridging the DVE lag on the last q
                    # tile's rotary; keeps the PE streak unbroken into the
                    # first scores matmul (HAM clock gate)
                    wup = ps_qkv.tile([128, 512], F32, tag="pqkv",
                                      name="wup")
                    for i in range(24):
                        nc.tensor.matmul(
                            wup[:], kT[0:64, 0, T - 128:T],
                            kT[0:64, 0, 0:512],
                            start=(i == 0), stop=(i == 23))
                    emit_tr(pend)

                # ================= Phase C: attention =================
                with (
                    tc.tile_pool(name="zp2", bufs=4) as zp2,
                    tc.tile_pool(name="nrm", bufs=2) as nrm,
                    tc.tile_pool(name="nrm1", bufs=1) as nrm1,
                    tc.tile_pool(name="ps_sc", bufs=2, space="PSUM") as ps_sc,
                    tc.tile_pool(name="ps_y", bufs=1, space="PSUM") as ps_y,
                ):
                    pend_tr = []    # q tiles awaiting transpose
                    pend_proj = []  # (rel, xcol) DMA'd, awaiting projection
                    for ch in range(NCH):
                        chs = slice(ch * CW, (ch + 1) * CW)
                        for h in range(HL):
                            rsl = slice((h % 2) * 64, (h % 2) * 64 + 64)
                            pr = h // 2
                            ya = ps_y.tile([65, CW], F32, tag="ya")
                            for s in range(NT):
                                ssl = slice(s * 128, (s + 1) * 128)
                                psc = ps_sc.tile([128, CW], F32, tag="psc")
                                for half in range(2):
                                    hsl = slice(half * 512, (half + 1) * 512)
                                    nc.tensor.matmul(
                                        psc[:, hsl], kT[rsl, pr, ssl],
                                        qTc[ch][rsl, pr, hsl],
                                        start=True, stop=True,
                                        tile_position=((h % 2) * 64, 0))
                                z = zp2.tile([128, CW], BF16, tag="z")
                                nc.scalar.activation(
                                    z[:, 0:EXPA], psc[:, 0:EXPA], AF.Exp,
                                    scale=0.125)
                                if EXPA < CW:
                                    nc.vector.tensor_scalar(
                                        z[:, EXPA:CW].bitcast(I16),
                                        psc[:, EXPA:CW],
                                        SCHRA, SCHRB, ALU.mult, ALU.add)
                                for half in range(2):
                                    hsl = slice(half * 512, (half + 1) * 512)
                                    nc.tensor.matmul(
                                        ya[:, hsl], vaug[:, s, h, :],
                                        z[:, hsl],
                                        start=(s == 0), stop=(s == NT - 1))
                            # normalize: evacuate psum, broadcast denom row,
                            # approx-reciprocal, scale into yTn
                            yu = nrm.tile([65, CW], F32, tag="yu")
                            nc.vector.tensor_copy(yu[:], ya[:])
                            dtmp = nrm1.tile([1, CW], F32, tag="dtmp")
                            nc.gpsimd.tensor_copy(dtmp[:], yu[64:65, :])
                            bc = nrm1.tile([128, CW], F32, tag="bc")
                            nc.gpsimd.partition_broadcast(bc[:], dtmp[:])
                            bcr = nrm1.tile([128, CW], F32, tag="bcr")
                            nc.vector.reciprocal_approx_fast(bcr[:], bc[:])
                            nc.vector.tensor_tensor(
                                yTn[rsl, pr, chs], yu[0:64, :],
                                bcr[0:64, :], op=ALU.mult)

                            # pipelined q tiles for the next chunk
                            if ch < NCH - 1:
                                base = 8 * (ch + 1)
                                for pend in pend_tr:
                                    emit_tr(pend)
                                pend_tr = []
                                nxt = []
                                for rel, xcol in pend_proj:
                                    if rel in PROJ_SCHED.get(h, []):
                                        ps_q = proj(xcol, "q", ps_tr)
                                        pend_tr.append(rotary_rms(
                                            base + rel, ps_q, qTc[ch + 1],
                                            rel * 128, cscr, csmall,
                                            on_act=False))
                                    else:
                                        nxt.append((rel, xcol))
                                pend_proj = nxt
                                for rel in DMA_SCHED.get(h, []):
                                    pend_proj.append(
                                        (rel, dma_xcol(base + rel)))
                        assert not pend_proj, (ch, pend_proj)

                # ============== tail: out-projection ==============
                with (
                    tc.tile_pool(name="ps_po", bufs=4, space="PSUM") as ps_po,
                    tc.tile_pool(name="ostg", bufs=4) as ostg,
                ):
                    # bridge burst across the last normalize drain
                    wdn = ps_po.tile([128, 512], F32, tag="po", name="wdn")
                    for i in range(12):
                        nc.tensor.matmul(
                            wdn[:], kT[0:64, 0, T - 128:T],
                            kT[0:64, 0, 0:512],
                            start=(i == 0), stop=(i == 11))
                    for ch_po in range(NCH):
                        for tt in range(8):
                            tsl = slice(ch_po * CW + tt * 128,
                                        ch_po * CW + (tt + 1) * 128)
                            for oc in range(2):
                                po = ps_po.tile([128, 512], F32, tag="po")
                                for kp in range(4):
                                    nc.tensor.matmul(
                                        po[:], yTn[:, kp, tsl],
                                        wo_sb[:, kp, oc * 512:(oc + 1) * 512],
                                        start=(kp == 0), stop=(kp == 3))
                                ost = ostg.tile([128, 512], F32, tag="ost")
                                nc.vector.tensor_copy(ost[:], po[:])
                                nc.sync.dma_start(
                                    out[tsl, oc * 512:(oc + 1) * 512], ost[:])

    nc.compile()
    return nc


_CACHE = {}


def _get_nc():
    if "nc" not in _CACHE:
        _CACHE["nc"] = build()
    return _CACHE["nc"]


def _prep_inputs(x, cos, sin, wq, wk, wv, wo):
    x = np.asarray(x, dtype=np.float32)
    cos = np.asarray(cos, dtype=np.float32).reshape(T, 32)
    sin = np.asarray(sin, dtype=np.float32).reshape(T, 32)
    wq = np.asarray(wq, dtype=np.float32)
    wk = np.asarray(wk, dtype=np.float32)
    wv = np.asarray(wv, dtype=np.float32)
    wo = np.asarray(wo, dtype=np.float32)

    cos2 = np.concatenate([cos, cos], axis=1)
    ss = np.concatenate([sin, -sin], axis=1)
    ident = np.eye(128, dtype=bf16)

    in_maps = []
    for c in range(8):
        b, hg = c // 2, c % 2
        rows = slice(hg * 512, (hg + 1) * 512)
        in_maps.append({
            "xT": np.ascontiguousarray(x[b].T).astype(bf16),
            "wqT": np.ascontiguousarray(wq[rows, :].T).astype(bf16),
            "wkT": np.ascontiguousarray(wk[rows, :].T).astype(bf16),
            "wvT": np.ascontiguousarray(wv[rows, :].T).astype(bf16),
            "woT": np.ascontiguousarray(wo[:, rows].T).astype(bf16),
            "cos2": cos2.astype(bf16),
            "ss": ss.astype(bf16),
            "ident": ident,
        })
    return in_maps


def _run(in_maps, trace=False):
    from concourse.bass_utils import run_bass_kernel_spmd

    nc = _get_nc()
    res = run_bass_kernel_spmd(nc, in_maps, core_ids=list(range(8)),
                               trace=trace)
    parts = [res.results[c]["out"] for c in range(8)]
    full = np.stack([parts[2 * b] + parts[2 * b + 1] for b in range(4)])
    return full.astype(np.float32), res


def kernel(x, cos, sin, wq, wk, wv, wo):
    in_maps = _prep_inputs(x, cos, sin, wq, wk, wv, wo)
    full, _ = _run(in_maps, trace=False)
    return full
